# revision 33
# baseline (speedup 1.0000x reference)
"""MultiHeadAttention Trainium2 kernel (8 NeuronCores, Bass/Tile).

Problem: B=2, S=2048, D=1024, H=16, DK=64 fp32 MHA (torch-Linear style
projections, softmax attention, output projection).

Sharding: core c = (batch b = c//4, head-group g = c%4); each core handles
4 heads of one batch, entirely in a transposed layout (features on
partitions, sequence on the free axis):
  qhT/khT  = (W_g x^T + b)       [2 pairs x 128, 2048]
  vh       = x_v Wv_g^T          [2048, 4x65] (ones col -> row sums)
  scoresT  = khT^T qhT           per (pair, ktile, qtile) -> PSUM
  expT     = exp(scoresT/8)      ACT -> bf16
  rawT     = vh_aug^T expT       PV matmul; row 64 = softmax denominator
  outT     = rawT[0:64] * (1/rawT[64])
  partial  = woT^T outT          [qt, 128, jt, 512] fp16 -> DRAM
Host: out[b] = sum_g partial(b,g) re-transposed + (Wo bv + bo).

Pipeline design (v2 baseline measured 212.5us; this version ~210us):
- The PE is the pacer: total real PE work is 164us (proj 41us, scores
  54.6us output-bound at K=64, PV 54.6us stream-bound at M=65, o-proj
  13.7us). ACT exp needs 139us and rides underneath. Measured PE busy
  ~182us (warmup/keepers/p-state on top of the 164), idle ~11us.
- x inputs are host-swizzled SEQ-MAJOR into 4 blocks of 512 positions
  ([128, blk, kt, 512], each (row, blk) an 8KB contiguous DRAM run).
  DMA runs as 4 parallel lanes, serial within each lane (a single ring
  only reaches ~50% of the 16-queue aggregate; a flat parallel pile
  delivers first-needed tensors at 1/N fair share — 4 need-ordered
  lanes get ~full aggregate AND early landing for wkb/xk_b0/wqb/xq_b0,
  all input done by ~50us). Lane-head configs are issued from the
  Activation sequencer (shorter preamble than SP; ~600ns/config).
- Projections are emitted as per-(pair, blk) quanta (8 matmuls + bias)
  chasing the per-block DMAs; k(0,b0)+q(0,b0) run up front, the rest
  ride as fillers in the scores stream. The Tile scheduler reorders by
  data readiness in its own cost-model sim — it typically bulk-runs
  the proj fillers before the stream settles, which keeps the PE
  continuously busy (that, not the emission interleave, is what
  matters: the kernel is PE-bound). Explicit dep edges to force the
  interleave were tried and cost ~40us (sequencer wait overhead +
  broken PE queue lookahead); a fully-serial DMA chain cost ~60us.
- Filler queue discipline vs the e2-slot SBUF reuse: all readers of a
  big-pool slot must be EMITTED before the e2 tile that recycles the
  slot is allocated (drain points after S(0,0) [xk], S(0,1) [xq],
  S(1,0) [xv]). One PV unit + one o-proj set per later window.
- Warmup: 12 N=512 matmuls carry the 0.65->2.4GHz p-state ramp, then
  75 N=128 fills keep the PE hot until xk_b0 lands (~18us).
- Tail: o-proj(3) pair-0 matmuls pre-issued into bank-aligned PSUM
  accs + zero-accumulate keepers bridge the final normalize; only the
  pair-1 matmuls + ACT copies + per-jt DMAs trail the last ot3 write.
  (A half-q split of the final unit was tried: the extra instruction
  overhead outweighed the shorter tail, 218us vs 210us.)
- partial output is fp16 with one 8KB-descriptor DMA per query tile
  (per-jt for the last tile); host sums the 4 head-group partials in
  fp32 and folds Wo@bv+bo.
- fp8 (DoubleRow) for scores/PV would halve the PE floor but fails the
  2e-2 gate (~3-4% quantization error vs 2.6e-3 measured at fp16/bf16;
  hi/lo compensation exactly cancels the throughput gain).
"""

import numpy as np

B, S, D, H = 2, 2048, 1024, 16
DK = D // H          # 64
N_CORES = 8
HG = H // 4          # 4 head-groups
HL = 4               # heads per core
FEAT = HL * DK       # 256 per-core features
NQT = S // 512       # 4 query tiles (= seq DMA blocks)
NKT = S // 128       # 16 key tiles
NDT = D // 128       # 8 contraction tiles (d-model)

DT_QK = "fp16"   # x_q/x_k, Wq/Wk, qhT/khT (score operands)
DT_V = "fp16"    # x_v, Wv
DT_PV = "bf16"   # vh_aug, expT
DT_O = "fp16"    # Wo, outT
N_WARMUP = 12    # PE p-state ramp matmuls (N=512) during initial DMA wait
N_WARMFILL = 120  # fine-grained (N=128) hold-hot matmuls until xk_b0 lands

_cache = {}


def _np_dt(name):
    if name == "fp16":
        return np.float16
    import ml_dtypes
    return ml_dtypes.bfloat16


def _build():
    import concourse.mybir as mybir
    import concourse.tile as tile
    from concourse import bacc

    fp32 = mybir.dt.float32
    dt_qk = getattr(mybir.dt, "float16" if DT_QK == "fp16" else "bfloat16")
    dt_v = getattr(mybir.dt, "float16" if DT_V == "fp16" else "bfloat16")
    dt_pv = getattr(mybir.dt, "float16" if DT_PV == "fp16" else "bfloat16")
    dt_o = getattr(mybir.dt, "float16" if DT_O == "fp16" else "bfloat16")
    dt_out = mybir.dt.float16

    nc = bacc.Bacc("TRN2", target_bir_lowering=False, debug=False,
                   num_devices=N_CORES)

    # x host-swizzled seq-major: [128, blk, kt, 512]; each (row, blk) is one
    # contiguous 8KB DRAM run -> per-block rings of 128 8KB descriptors.
    # qk biases ride in the qk weight buffer (fp32 bit-packed into 4
    # trailing fp16 columns, bitcast on device) to avoid tiny descriptors.
    xqT = nc.dram_tensor("xqT", [128, NQT * NDT * 512], dt_qk,
                         kind="ExternalInput").ap()
    xkT = nc.dram_tensor("xkT", [128, NQT * NDT * 512], dt_qk,
                         kind="ExternalInput").ap()
    xvT = nc.dram_tensor("xvT", [128, NQT * NDT * 512], dt_v,
                         kind="ExternalInput").ap()
    wkbT = nc.dram_tensor("wkbT", [128, NDT * FEAT + 4], dt_qk,
                          kind="ExternalInput").ap()
    wqbT = nc.dram_tensor("wqbT", [128, NDT * FEAT + 4], dt_qk,
                          kind="ExternalInput").ap()
    wvoT = nc.dram_tensor("wvoT", [128, NDT * FEAT + 2 * D], dt_v,
                          kind="ExternalInput").ap()
    # output layout [qt, p, jt, s]: each partition row is one contiguous
    # 8KB write (fewer, bigger DMA descriptors); host re-transposes
    out_d = nc.dram_tensor("partialT", [NQT, 128, NDT, 512], dt_out,
                           kind="ExternalOutput").ap()

    xq_r = xqT.rearrange("p (b t s) -> p b t s", b=NQT, t=NDT)
    xk_r = xkT.rearrange("p (b t s) -> p b t s", b=NQT, t=NDT)
    xv_r = xvT.rearrange("p (b t s) -> p b t s", b=NQT, t=NDT)

    with tile.TileContext(nc) as tc:
        def chain(inst, key):
            # serial DMA ring chain: ring N+1 starts only after ring N
            # completes, so first-needed tensors get the full HBM link.
            # (chain_iter_dep takes the raw mybir.Instruction — passing the
            # BassInstruction wrapper raises, which a silent try/except hid
            # in earlier versions: the rings actually ran in parallel.)
            tc.chain_iter_dep(key, inst.ins)

        with (
            tc.tile_pool(name="win", bufs=1) as win,
            tc.tile_pool(name="big", bufs=4) as big,
            tc.tile_pool(name="proj", bufs=1) as proj,
            tc.tile_pool(name="pout", bufs=1) as pout,
            tc.tile_pool(name="pnrm", bufs=2) as pnrm,
            tc.tile_pool(name="pp", bufs=2, space="PSUM") as pp,
            tc.tile_pool(name="ps2", bufs=2, space="PSUM") as ps2,
            tc.tile_pool(name="pspv", bufs=2, space="PSUM") as pspv,
        ):
            wdum0 = win.tile([128, 512], dt_qk, tag="wdum")
            junk = win.tile([128, 512], dt_qk, tag="junk")
            nc.vector.memset(wdum0[:], 0.0)
            wkb = win.tile([128, NDT * FEAT + 4], dt_qk, tag="wkb")
            wqb = win.tile([128, NDT * FEAT + 4], dt_qk, tag="wqb")
            wvo = win.tile([128, NDT * FEAT + 2 * D], dt_v, tag="wvo")

            xk3 = big.tile([128, NQT, NDT, 512], dt_qk, tag="big")
            xq3 = big.tile([128, NQT, NDT, 512], dt_qk, tag="big")
            xv3 = big.tile([128, NQT, NDT, 512], dt_v, tag="big")

            # ---- DMA: 4 parallel lanes, serial within each lane. One ring
            # alone only reaches ~50% of the 16-queue aggregate, while a
            # flat parallel pile delivers first-needed tensors at 1/N fair
            # share — 4 concurrent need-ordered lanes get both: ~full
            # aggregate AND early landing for wkb/xk_b0/wqb/xq_b0. Lane
            # heads are configured on the Activation sequencer (shorter
            # preamble than SP, idle until the exp stream). ----
            chain(nc.scalar.dma_start(wkb[:], wkbT), "l0")
            chain(nc.scalar.dma_start(xk3[:, 0], xk_r[:, 0]), "l1")
            chain(nc.scalar.dma_start(wqb[:], wqbT), "l2")
            chain(nc.scalar.dma_start(xq3[:, 0], xq_r[:, 0]), "l3")
            nc.scalar.activation(junk[0:1, 0:1], wdum0[0:1, 0:1],
                                 mybir.ActivationFunctionType.Exp, scale=1.0)
            chain(nc.sync.dma_start(xk3[:, 1], xk_r[:, 1]), "l0")
            chain(nc.sync.dma_start(xk3[:, 2], xk_r[:, 2]), "l1")
            chain(nc.sync.dma_start(xk3[:, 3], xk_r[:, 3]), "l2")
            chain(nc.sync.dma_start(xq3[:, 1], xq_r[:, 1]), "l3")
            chain(nc.sync.dma_start(xq3[:, 2], xq_r[:, 2]), "l0")
            chain(nc.sync.dma_start(xq3[:, 3], xq_r[:, 3]), "l1")
            chain(nc.sync.dma_start(wvo[:], wvoT), "l2")
            chain(nc.sync.dma_start(xv3[:, 3], xv_r[:, 3]), "l3")
            chain(nc.sync.dma_start(xv3[:, 0], xv_r[:, 0]), "l0")
            chain(nc.sync.dma_start(xv3[:, 1], xv_r[:, 1]), "l1")
            chain(nc.sync.dma_start(xv3[:, 2], xv_r[:, 2]), "l2")

            wk3 = wkb[:, 0:NDT * FEAT].rearrange("p (t f) -> p t f", t=NDT)
            wq3 = wqb[:, 0:NDT * FEAT].rearrange("p (t f) -> p t f", t=NDT)
            bk3 = wkb[:, NDT * FEAT:NDT * FEAT + 4].bitcast(fp32)
            bq3 = wqb[:, NDT * FEAT:NDT * FEAT + 4].bitcast(fp32)
            wv3 = wvo[:, 0:NDT * FEAT].rearrange("p (t f) -> p t f", t=NDT)
            wo3 = wvo[:, NDT * FEAT:].rearrange("p (t j) -> p t j", t=2)

            # ---- persistent intermediates ----
            qh3 = proj.tile([128, 2, S], dt_qk, tag="qh")   # pair-packed
            kh3 = proj.tile([128, 2, S], dt_qk, tag="kh")
            vha = proj.tile([128, NKT, HL, DK + 1], dt_pv, tag="vha")
            ot3 = proj.tile([128, 2, S], dt_o, tag="outT")
            nc.gpsimd.memset(vha[:, :, :, DK], 1.0)  # ones col -> denominators

            # ---- PE p-state warmup while the first DMAs land: big matmuls
            # carry the clock ramp, then fine-grained N=128 fills keep the
            # PE hot (and the overshoot cheap) until xk_b0/wkb arrive ----
            wdum = wdum0
            wu = pp.tile([128, 512], fp32, tag="acc")
            for i in range(N_WARMUP):
                nc.tensor.matmul(wu[:], wdum[:, 0:128], wdum[:],
                                 start=(i == 0), stop=(i == N_WARMUP - 1))
            for i in range(N_WARMFILL):
                nc.tensor.matmul(wu[:, 0:128], wdum[:, 0:128],
                                 wdum[:, 0:128], start=True, stop=True)
            nc.vector.tensor_copy(junk[:], wu[:])

            # ---- projection quanta: one (pair m, seq-block blk) at a time,
            # kt-inner, chasing the per-block x DMAs ----
            def qk_blk(x3, w3, b3, dst, m, blk):
                acc = pp.tile([128, 512], fp32, tag="acc", name="acc")
                first = None
                for kt in range(NDT):
                    i = nc.tensor.matmul(
                        acc[:], w3[:, kt, m * 128:(m + 1) * 128],
                        x3[:, blk, kt, :],
                        start=(kt == 0), stop=(kt == NDT - 1))
                    first = first or i
                nc.vector.tensor_scalar_add(
                    dst[:, m, blk * 512:(blk + 1) * 512], acc[:],
                    b3[:, m:m + 1])
                return first

            def v_quantum(st):
                ps = pp.tile([128, 512], fp32, tag="acc", name="vacc")
                first = None
                for kt in range(NDT):
                    i = nc.tensor.matmul(
                        ps[:, 0:256],
                        xv3[:, st // 4, kt, (st % 4) * 128:(st % 4 + 1) * 128],
                        wv3[:, kt, :],
                        start=(kt == 0), stop=(kt == NDT - 1))
                    first = first or i
                nc.vector.tensor_copy(vha[:, st, :, 0:DK], ps[:, 0:256])
                return first

            def pv_quantum(state, qt, hp, e2u, kt):
                if "a" not in state:
                    state["a"] = pspv.tile([DK + 1, 512], fp32, tag="pv",
                                           name="pva")
                    state["b"] = pspv.tile([DK + 1, 512], fp32, tag="pv",
                                           name="pvb")
                i = nc.tensor.matmul(
                    state["a"][:], vha[:, kt, 2 * hp, :], e2u[:, kt, 0:512],
                    start=(kt == 0), stop=(kt == NKT - 1))
                nc.tensor.matmul(
                    state["b"][:], vha[:, kt, 2 * hp + 1, :],
                    e2u[:, kt, 512:1024],
                    start=(kt == 0), stop=(kt == NKT - 1))
                return i

            def norm(state, qt, hp, direct=False):
                # whole-accumulator copy frees the PSUM bank early; custom
                # DVE recip needs a base-partition-0 SBUF input (srow).
                # direct=True (final unit): skip the copy, read PSUM in
                # place — shorter critical chain, the bank isn't needed.
                for pv, half in ((state["a"], 0), (state["b"], 1)):
                    if direct:
                        pvs = pv
                    else:
                        pvs = pnrm.tile([DK + 1, 512], fp32, tag="pvs")
                        nc.vector.tensor_copy(pvs[:], pv[:])
                    srow = pnrm.tile([1, 512], fp32, tag="srow")
                    nc.vector.tensor_copy(srow[:], pvs[DK:DK + 1, :])
                    inv = pnrm.tile([1, 512], fp32, tag="inv")
                    nc.vector.reciprocal_approx_fast(inv[:], srow[:])
                    invb = pnrm.tile([64, 512], fp32, tag="invb")
                    nc.gpsimd.partition_broadcast(invb[:], inv[:])
                    nc.vector.tensor_tensor(
                        ot3[half * 64:(half + 1) * 64, hp,
                            qt * 512:(qt + 1) * 512],
                        pvs[0:DK, :], invb[:], mybir.AluOpType.mult)

            def oproj_quantum(pstate, qt, jt):
                if "po" not in pstate:
                    pstate["po"] = pout.tile([128, NDT, 512], dt_out,
                                             tag="po", bufs=1, name="po")
                ps = pp.tile([128, 512], fp32, tag="acc", name="oacc")
                first = None
                for m in range(2):
                    i = nc.tensor.matmul(
                        ps[:], wo3[:, m, jt * 128:(jt + 1) * 128],
                        ot3[:, m, qt * 512:(qt + 1) * 512],
                        start=(m == 0), stop=(m == 1))
                    first = first or i
                if qt == NQT - 1:
                    # ACT is idle once the exp stream ends; casting there
                    # overlaps the DVE normalize chain
                    nc.scalar.copy(pstate["po"][:, jt, :], ps[:])
                else:
                    nc.vector.tensor_copy(pstate["po"][:, jt, :], ps[:])
                if qt == NQT - 1:
                    # last tile: per-jt DMA starts the final drain earlier
                    nc.sync.dma_start(out_d[qt, :, jt:jt + 1, :],
                                      pstate["po"][:, jt:jt + 1, :])
                elif jt == NDT - 1:
                    nc.sync.dma_start(out_d[qt], pstate["po"][:])
                return first

            def e2tile(name):
                return big.tile([128, NKT, 1024], dt_pv, tag="big", name=name)

            # ---- filler queue: PE work that rides in the slack of the
            # exp-paced scores stream (ACT needs ~1088ns/kt, scores only
            # ~426ns of PE) so the exp stream never starves. Each released
            # filler's first matmul is dep-chained to the preceding scores
            # matmul: the Tile scheduler orders by its own cost model, and
            # without the explicit dep it hoists ready fillers wholesale
            # ahead of blocked scores (v3 lost 12us to exactly that). ----
            fillers = []      # list of (cost_ns, thunk)
            fq = {"i": 0, "budget": 0.0, "n": 0, "last": None}

            def pace(first_inst):
                # No explicit ordering edge: the Tile scheduler orders by
                # data readiness in its own cost-model sim, and measured
                # schedules show it interleaving fillers with the scores
                # stream near-optimally on its own. Explicit dep edges to
                # the last scores matmul were tried and cost ~40us: each
                # edge adds sequencer wait overhead and breaks the PE
                # queue's lookahead pipelining.
                pass

            def release_next():
                cost, fn = fillers[fq["i"]]
                fq["i"] += 1
                pace(fn())

            def drain_fillers():
                while fq["i"] < len(fillers):
                    release_next()

            SLACK_NS = 900    # filler budget added per exp-paced kt step

            def s_unit(qt, hp, e2u):
                for kt in range(NKT):
                    s2 = ps2.tile([128, 1024], fp32, tag="s2")
                    nc.tensor.matmul(
                        s2[:, 0:512],
                        kh3[0:64, hp, kt * 128:(kt + 1) * 128],
                        qh3[0:64, hp, qt * 512:(qt + 1) * 512],
                        start=True, stop=True)
                    fq["last"] = nc.tensor.matmul(
                        s2[:, 512:1024],
                        kh3[64:128, hp, kt * 128:(kt + 1) * 128],
                        qh3[64:128, hp, qt * 512:(qt + 1) * 512],
                        start=True, stop=True)
                    nc.scalar.activation(
                        e2u[:, kt, :], s2[:],
                        mybir.ActivationFunctionType.Exp, scale=0.125)
                    fq["budget"] += SLACK_NS
                    while (fq["i"] < len(fillers)
                           and fillers[fq["i"]][0] <= fq["budget"]):
                        fq["budget"] -= fillers[fq["i"]][0]
                        release_next()

            # ---- emission: k(0,b0)+q(0,b0) up front, then the exp-paced
            # score stream with everything else as fillers. The k fillers
            # alternate pair-1 (xk_b0-resident, ready immediately) with
            # pair-0 blocks (chasing the lane DMAs); each k(0,b) releases
            # BEFORE the scores step that reads it — both for correctness
            # of the pace() edge (no cycle) and so the stream only stalls
            # on the DMA itself. Queue order respects the e2-slot reuse
            # deps (all xk readers emitted before e01 takes xk3's slot,
            # xq readers before e10, xv readers before e11). ----
            qk_blk(xk3, wk3, bk3, kh3, 0, 0)
            qk_blk(xq3, wq3, bq3, qh3, 0, 0)

            for m, blk in ((1, 0), (0, 1), (1, 1), (0, 2), (1, 2), (0, 3),
                           (1, 3), (None, None)):
                if m is None:
                    fillers.append(
                        (1750, lambda: qk_blk(xq3, wq3, bq3, qh3, 1, 0)))
                else:
                    fillers.append(
                        (1750, lambda m=m, b=blk: qk_blk(
                            xk3, wk3, bk3, kh3, m, b)))

            e00 = e2tile("e00")
            s_unit(0, 0, e00)                        # ACT starts here
            drain_fillers()                          # xk readers, see above
            for m, blk in ((0, 1), (1, 1), (0, 2), (1, 2), (0, 3), (1, 3)):
                fillers.append(
                    (1750, lambda m=m, b=blk: qk_blk(
                        xq3, wq3, bq3, qh3, m, b)))
            e01 = e2tile("e01")
            s_unit(0, 1, e01)
            drain_fillers()                          # xq readers
            for st in range(NKT):                    # v-proj -> fillers
                fillers.append((900, lambda st=st: v_quantum(st)))
            e10 = e2tile("e10")
            s_unit(1, 0, e10)
            drain_fillers()                          # xv readers
            pv_states = {}
            for u, (uq, uh, eu) in enumerate(((0, 0, e00), (0, 1, e01))):
                st_ = pv_states[(uq, uh)] = {}
                for kt in range(NKT):
                    fillers.append(
                        (440, lambda s=st_, q=uq, h=uh, e=eu, k=kt:
                         pv_quantum(s, q, h, e, k)))
                fillers.append(
                    (0, lambda s=st_, q=uq, h=uh: norm(s, q, h)))
            e11 = e2tile("e11")
            s_unit(1, 1, e11)

            prev = {(1, 0): e10, (1, 1): e11}
            def queue_pv(qt, hp):
                st_ = pv_states[(qt, hp)] = {}
                eu = prev[(qt, hp)]
                for kt in range(NKT):
                    fillers.append(
                        (440, lambda s=st_, q=qt, h=hp, e=eu, k=kt:
                         pv_quantum(s, q, h, e, k)))
                fillers.append(
                    (0, lambda s=st_, q=qt, h=hp: norm(s, q, h)))

            def queue_oproj(qt):
                pstate = {}
                for jt in range(NDT):
                    fillers.append(
                        (480, lambda p=pstate, q=qt, j=jt:
                         oproj_quantum(p, q, j)))

            # one PV unit (+ one oproj set) per remaining window, so the
            # filler queue inflow matches the ~14us/window release budget
            queue_pv(1, 0)
            queue_pv(1, 1)
            e20 = e2tile("e20")
            prev[(2, 0)] = e20
            s_unit(2, 0, e20)
            queue_pv(2, 0)
            queue_oproj(0)
            e21 = e2tile("e21")
            prev[(2, 1)] = e21
            s_unit(2, 1, e21)
            queue_pv(2, 1)
            queue_oproj(1)
            e30 = e2tile("e30")
            prev[(3, 0)] = e30
            s_unit(3, 0, e30)
            queue_pv(3, 0)
            queue_oproj(2)
            e31 = e2tile("e31")
            prev[(3, 1)] = e31
            s_unit(3, 1, e31)

            # tail: leftover fillers, last PV tracking the last exps, then
            # o-proj(3) pair-0 matmuls pre-issued (plus zero-accumulate
            # keepers) bridge the final normalize so the PE stays hot and
            # only the pair-1 matmuls trail the last ot3 write
            drain_fillers()
            st_ = pv_states[(3, 1)] = {}
            for kt in range(NKT):
                pv_quantum(st_, 3, 1, prev[(3, 1)], kt)
            q3s = slice((NQT - 1) * 512, NQT * 512)
            accs = []
            for jt in range(6):
                if jt % 2 == 0 and jt < 4:
                    ot_ps = ps2.tile([128, 1024], fp32, tag="s2",
                                     name=f"otps{jt // 2}")
                if jt < 4:
                    acc = ot_ps[:, (jt % 2) * 512:(jt % 2 + 1) * 512]
                else:
                    acc = pp.tile([128, 512], fp32, tag="acc",
                                  name=f"oacc{jt}")[:]
                nc.tensor.matmul(acc, wo3[:, 0, jt * 128:(jt + 1) * 128],
                                 ot3[:, 0, q3s], start=True, stop=False)
                accs.append(acc)
            norm(st_, 3, 1)
            # zero-accumulate keepers during the norm; extra rounds on the
            # pp-based accs (jt 4/5) — those have no PSUM-slot WAR on the
            # last exps, so the scheduler can actually place them in the
            # normalize window (the ps2-based ones resolve late in its sim)
            for acc in accs:
                nc.tensor.matmul(acc, wdum[:, 0:128], wdum[:],
                                 start=False, stop=False)
            for _ in range(5):
                for acc in accs[4:6]:
                    nc.tensor.matmul(acc, wdum[:, 0:128], wdum[:],
                                     start=False, stop=False)
            po = pout.tile([128, NDT, 512], dt_out, tag="po", bufs=1,
                           name="po")
            for jt in range(6):
                nc.tensor.matmul(accs[jt],
                                 wo3[:, 1, jt * 128:(jt + 1) * 128],
                                 ot3[:, 1, q3s], start=False, stop=True)
                if jt % 2 == 0:
                    nc.vector.tensor_copy(po[:, jt, :], accs[jt])
                else:
                    nc.scalar.copy(po[:, jt, :], accs[jt])
                nc.sync.dma_start(out_d[NQT - 1, :, jt:jt + 1, :],
                                  po[:, jt:jt + 1, :])
            pstate = {"po": po}
            for jt in (6, 7):
                oproj_quantum(pstate, 3, jt)

    nc.compile()
    return nc


def kernel(q, k, v, Wq, bq, Wk, bk, Wv, bv, Wo, bo, _trace=False):
    from concourse import bass_utils

    if "nc" not in _cache:
        _cache["nc"] = _build()
    nc = _cache["nc"]

    q = np.asarray(q, np.float32)
    k = np.asarray(k, np.float32)
    v = np.asarray(v, np.float32)
    Wq = np.asarray(Wq, np.float32)
    Wk = np.asarray(Wk, np.float32)
    Wv = np.asarray(Wv, np.float32)
    Wo = np.asarray(Wo, np.float32)
    bq = np.asarray(bq, np.float32)
    bk = np.asarray(bk, np.float32)
    bv = np.asarray(bv, np.float32)
    bo = np.asarray(bo, np.float32)

    d_qk, d_v, d_o = _np_dt(DT_QK), _np_dt(DT_V), _np_dt(DT_O)

    def swz(a, t):     # [t*128, f] -> [128, t*f], rows contiguous in DRAM
        f = a.shape[1]
        return np.ascontiguousarray(
            a.reshape(t, 128, f).transpose(1, 0, 2).reshape(128, t * f))

    def swz_seq(a):    # [D, S] -> [128, (blk, kt, 512)] seq-major blocks
        # a[t*128+p, blk*512+s] -> out[p, ((blk*NDT)+t)*512+s]
        return np.ascontiguousarray(
            a.reshape(NDT, 128, NQT, 512).transpose(1, 2, 0, 3)
            .reshape(128, NQT * NDT * 512))

    xT = {}
    for b in range(B):
        xT[("q", b)] = swz_seq(np.ascontiguousarray(q[b].T)).astype(d_qk)
        xT[("k", b)] = swz_seq(np.ascontiguousarray(k[b].T)).astype(d_qk)
        xT[("v", b)] = swz_seq(np.ascontiguousarray(v[b].T)).astype(d_v)
    wT = {}
    for g in range(HG):
        sl = slice(g * FEAT, (g + 1) * FEAT)
        wk_s = swz(np.ascontiguousarray(Wk[sl, :].T), NDT).astype(d_qk)
        wq_s = swz(np.ascontiguousarray(Wq[sl, :].T), NDT).astype(d_qk)
        bk_s = np.ascontiguousarray(
            bk[sl].astype(np.float32).reshape(2, 128).T).view(np.uint16)
        bq_s = np.ascontiguousarray(
            bq[sl].astype(np.float32).reshape(2, 128).T).view(np.uint16)
        wv_s = swz(np.ascontiguousarray(Wv[sl, :].T), NDT).astype(d_v)
        wo_s = swz(np.ascontiguousarray(Wo[:, sl].T), 2).astype(d_o)
        wT[("kb", g)] = np.ascontiguousarray(np.concatenate(
            [wk_s.view(np.uint16), bk_s], axis=1).view(d_qk))
        wT[("qb", g)] = np.ascontiguousarray(np.concatenate(
            [wq_s.view(np.uint16), bq_s], axis=1).view(d_qk))
        wT[("vo", g)] = np.ascontiguousarray(
            np.concatenate([wv_s, wo_s], axis=1))

    in_maps = []
    for c in range(N_CORES):
        b, g = divmod(c, HG)
        in_maps.append({
            "xqT": xT[("q", b)], "xkT": xT[("k", b)], "xvT": xT[("v", b)],
            "wkbT": wT[("kb", g)], "wqbT": wT[("qb", g)],
            "wvoT": wT[("vo", g)],
        })

    kwargs = {}
    if _trace:
        _install_profile_shim()
        kwargs = dict(trace=True, trace_cores=list(range(N_CORES)))
    res = bass_utils.run_bass_kernel_spmd(
        nc, in_maps, core_ids=list(range(N_CORES)), **kwargs)
    _cache["last_results"] = res

    final_bias = (Wo @ bv + bo).astype(np.float32)  # attn rows sum to 1
    out = np.empty((B, S, D), np.float32)
    for b in range(B):
        acc = res.results[b * HG]["partialT"].astype(np.float32)
        for g in range(1, HG):
            acc += res.results[b * HG + g]["partialT"].astype(np.float32)
        # [qt, p, jt, s] -> [S, D]:  d = jt*128+p, q = qt*512+s
        out[b] = acc.transpose(0, 3, 2, 1).reshape(S, D) + final_bias
    return out


def _install_profile_shim():
    """Provide antenv.axon_hooks so trace=True works under axon."""
    import sys
    import types

    import antenv

    if "antenv.axon_hooks" in sys.modules:
        return
    mod = types.ModuleType("antenv.axon_hooks")
    mod._hook = None
    mod.set_axon_ntff_profile_hook = lambda h: setattr(mod, "_hook", h)
    mod.get_axon_ntff_profile_hook = lambda: mod._hook
    sys.modules["antenv.axon_hooks"] = mod
    antenv.axon_hooks = mod
    try:
        from trn_agent_boot.trn_boot import _ntff_profile_via_ctypes
        mod.set_axon_ntff_profile_hook(
            _ntff_profile_via_ctypes("/opt/axon/libaxon_pjrt.so"))
    except Exception:
        pass


# revision 37
# speedup vs baseline: 1.0095x; 1.0095x over previous
"""MultiHeadAttention Trainium2 kernel (8 NeuronCores, Bass/Tile).

Problem: B=2, S=2048, D=1024, H=16, DK=64 fp32 MHA (torch-Linear style
projections, softmax attention, output projection).

Sharding: core c = (batch b = c//4, head-group g = c%4); each core handles
4 heads of one batch, entirely in a transposed layout (features on
partitions, sequence on the free axis):
  qhT/khT  = (W_g x^T + b)       [2 pairs x 128, 2048]
  vh       = x_v Wv_g^T          [2048, 4x65] (ones col -> row sums)
  scoresT  = khT^T qhT           per (pair, ktile, qtile) -> PSUM
  expT     = exp(scoresT/8)      ACT -> bf16
  rawT     = vh_aug^T expT       PV matmul; row 64 = softmax denominator
  outT     = rawT[0:64] * (1/rawT[64])
  partial  = woT^T outT          [qt, 128, jt, 512] fp16 -> DRAM
Host: out[b] = sum_g partial(b,g) re-transposed + (Wo bv + bo).

Pipeline design (v2 baseline measured 212.5us; this version ~210us):
- The PE is the pacer: total real PE work is 164us (proj 41us, scores
  54.6us output-bound at K=64, PV 54.6us stream-bound at M=65, o-proj
  13.7us). ACT exp needs 139us and rides underneath. Measured PE busy
  ~182us (warmup/keepers/p-state on top of the 164), idle ~11us.
- x inputs are host-swizzled SEQ-MAJOR into 4 blocks of 512 positions
  ([128, blk, kt, 512], each (row, blk) an 8KB contiguous DRAM run).
  DMA runs as 4 parallel lanes, serial within each lane (a single ring
  only reaches ~50% of the 16-queue aggregate; a flat parallel pile
  delivers first-needed tensors at 1/N fair share — 4 need-ordered
  lanes get ~full aggregate AND early landing for wkb/xk_b0/wqb/xq_b0,
  all input done by ~50us). Lane-head configs are issued from the
  Activation sequencer (shorter preamble than SP; ~600ns/config).
- Projections are emitted as per-(pair, blk) quanta (8 matmuls + bias)
  chasing the per-block DMAs; k(0,b0)+q(0,b0) run up front, the rest
  ride as fillers in the scores stream. The Tile scheduler reorders by
  data readiness in its own cost-model sim — it typically bulk-runs
  the proj fillers before the stream settles, which keeps the PE
  continuously busy (that, not the emission interleave, is what
  matters: the kernel is PE-bound). Explicit dep edges to force the
  interleave were tried and cost ~40us (sequencer wait overhead +
  broken PE queue lookahead); a fully-serial DMA chain cost ~60us.
- Filler queue discipline vs the e2-slot SBUF reuse: all readers of a
  big-pool slot must be EMITTED before the e2 tile that recycles the
  slot is allocated (drain points after S(0,0) [xk], S(0,1) [xq],
  S(1,0) [xv]). One PV unit + one o-proj set per later window.
- Warmup: 12 N=512 matmuls carry the 0.65->2.4GHz p-state ramp, then
  75 N=128 fills keep the PE hot until xk_b0 lands (~18us).
- Tail: o-proj(3) pair-0 matmuls pre-issued into bank-aligned PSUM
  accs + zero-accumulate keepers bridge the final normalize; only the
  pair-1 matmuls + ACT copies + per-jt DMAs trail the last ot3 write.
  (A half-q split of the final unit was tried: the extra instruction
  overhead outweighed the shorter tail, 218us vs 210us.)
- partial output is fp16 with one 8KB-descriptor DMA per query tile
  (per-jt for the last tile); host sums the 4 head-group partials in
  fp32 and folds Wo@bv+bo.
- fp8 (DoubleRow) for scores/PV would halve the PE floor but fails the
  2e-2 gate (~3-4% quantization error vs 2.6e-3 measured at fp16/bf16;
  hi/lo compensation exactly cancels the throughput gain).
"""

import numpy as np

B, S, D, H = 2, 2048, 1024, 16
DK = D // H          # 64
N_CORES = 8
HG = H // 4          # 4 head-groups
HL = 4               # heads per core
FEAT = HL * DK       # 256 per-core features
NQT = S // 512       # 4 query tiles (= seq DMA blocks)
NKT = S // 128       # 16 key tiles
NDT = D // 128       # 8 contraction tiles (d-model)

DT_QK = "fp16"   # x_q/x_k, Wq/Wk, qhT/khT (score operands)
DT_V = "fp16"    # x_v, Wv
DT_PV = "bf16"   # vh_aug, expT
DT_O = "fp16"    # Wo, outT
N_WARMUP = 12    # PE p-state ramp matmuls (N=512) during initial DMA wait
N_WARMFILL = 120  # fine-grained (N=128) hold-hot matmuls until xk_b0 lands

_cache = {}


def _np_dt(name):
    if name == "fp16":
        return np.float16
    import ml_dtypes
    return ml_dtypes.bfloat16


def _build():
    import concourse.mybir as mybir
    import concourse.tile as tile
    from concourse import bacc

    fp32 = mybir.dt.float32
    dt_qk = getattr(mybir.dt, "float16" if DT_QK == "fp16" else "bfloat16")
    dt_v = getattr(mybir.dt, "float16" if DT_V == "fp16" else "bfloat16")
    dt_pv = getattr(mybir.dt, "float16" if DT_PV == "fp16" else "bfloat16")
    dt_o = getattr(mybir.dt, "float16" if DT_O == "fp16" else "bfloat16")
    dt_out = mybir.dt.float16

    nc = bacc.Bacc("TRN2", target_bir_lowering=False, debug=False,
                   num_devices=N_CORES)

    # x host-swizzled seq-major: [128, blk, kt, 512]; each (row, blk) is one
    # contiguous 8KB DRAM run -> per-block rings of 128 8KB descriptors.
    # qk biases ride in the qk weight buffer (fp32 bit-packed into 4
    # trailing fp16 columns, bitcast on device) to avoid tiny descriptors.
    xqT = nc.dram_tensor("xqT", [128, NQT * NDT * 512], dt_qk,
                         kind="ExternalInput").ap()
    xkT = nc.dram_tensor("xkT", [128, NQT * NDT * 512], dt_qk,
                         kind="ExternalInput").ap()
    xvT = nc.dram_tensor("xvT", [128, NQT * NDT * 512], dt_v,
                         kind="ExternalInput").ap()
    wkbT = nc.dram_tensor("wkbT", [128, NDT * FEAT + 4], dt_qk,
                          kind="ExternalInput").ap()
    wqbT = nc.dram_tensor("wqbT", [128, NDT * FEAT + 4], dt_qk,
                          kind="ExternalInput").ap()
    wvoT = nc.dram_tensor("wvoT", [128, NDT * FEAT + 2 * D], dt_v,
                          kind="ExternalInput").ap()
    # output layout [qt, p, jt, s]: each partition row is one contiguous
    # 8KB write (fewer, bigger DMA descriptors); host re-transposes
    out_d = nc.dram_tensor("partialT", [NQT, 128, NDT, 512], dt_out,
                           kind="ExternalOutput").ap()

    xq_r = xqT.rearrange("p (b t s) -> p b t s", b=NQT, t=NDT)
    xk_r = xkT.rearrange("p (b t s) -> p b t s", b=NQT, t=NDT)
    xv_r = xvT.rearrange("p (b t s) -> p b t s", b=NQT, t=NDT)

    with tile.TileContext(nc) as tc:
        def chain(inst, key):
            # serial DMA ring chain: ring N+1 starts only after ring N
            # completes, so first-needed tensors get the full HBM link.
            # (chain_iter_dep takes the raw mybir.Instruction — passing the
            # BassInstruction wrapper raises, which a silent try/except hid
            # in earlier versions: the rings actually ran in parallel.)
            tc.chain_iter_dep(key, inst.ins)

        with (
            tc.tile_pool(name="win", bufs=1) as win,
            tc.tile_pool(name="big", bufs=4) as big,
            tc.tile_pool(name="proj", bufs=1) as proj,
            tc.tile_pool(name="pout", bufs=1) as pout,
            tc.tile_pool(name="pnrm", bufs=2) as pnrm,
            tc.tile_pool(name="pp", bufs=2, space="PSUM") as pp,
            tc.tile_pool(name="ps2", bufs=2, space="PSUM") as ps2,
            tc.tile_pool(name="pspv", bufs=2, space="PSUM") as pspv,
        ):
            wdum0 = win.tile([128, 512], dt_qk, tag="wdum")
            junk = win.tile([128, 512], dt_qk, tag="junk")
            nc.vector.memset(wdum0[:], 0.0)
            wkb = win.tile([128, NDT * FEAT + 4], dt_qk, tag="wkb")
            wqb = win.tile([128, NDT * FEAT + 4], dt_qk, tag="wqb")
            wvo = win.tile([128, NDT * FEAT + 2 * D], dt_v, tag="wvo")

            xk3 = big.tile([128, NQT, NDT, 512], dt_qk, tag="big")
            xq3 = big.tile([128, NQT, NDT, 512], dt_qk, tag="big")
            xv3 = big.tile([128, NQT, NDT, 512], dt_v, tag="big")

            # ---- DMA: 4 parallel lanes, serial within each lane. One ring
            # alone only reaches ~50% of the 16-queue aggregate, while a
            # flat parallel pile delivers first-needed tensors at 1/N fair
            # share — 4 concurrent need-ordered lanes get both: ~full
            # aggregate AND early landing for wkb/xk_b0/wqb/xq_b0. Lane
            # heads are configured on the Activation sequencer (shorter
            # preamble than SP, idle until the exp stream). ----
            chain(nc.scalar.dma_start(wkb[:], wkbT), "l0")
            chain(nc.scalar.dma_start(xk3[:, 0], xk_r[:, 0]), "l1")
            chain(nc.scalar.dma_start(wqb[:], wqbT), "l2")
            chain(nc.scalar.dma_start(xq3[:, 0], xq_r[:, 0]), "l3")
            nc.scalar.activation(junk[0:1, 0:1], wdum0[0:1, 0:1],
                                 mybir.ActivationFunctionType.Exp, scale=1.0)
            chain(nc.sync.dma_start(xk3[:, 1], xk_r[:, 1]), "l0")
            chain(nc.sync.dma_start(xk3[:, 2], xk_r[:, 2]), "l1")
            chain(nc.sync.dma_start(xk3[:, 3], xk_r[:, 3]), "l2")
            chain(nc.sync.dma_start(xq3[:, 1], xq_r[:, 1]), "l3")
            chain(nc.sync.dma_start(xq3[:, 2], xq_r[:, 2]), "l0")
            chain(nc.sync.dma_start(xq3[:, 3], xq_r[:, 3]), "l1")
            chain(nc.sync.dma_start(wvo[:], wvoT), "l2")
            chain(nc.sync.dma_start(xv3[:, 3], xv_r[:, 3]), "l3")
            chain(nc.sync.dma_start(xv3[:, 0], xv_r[:, 0]), "l0")
            chain(nc.sync.dma_start(xv3[:, 1], xv_r[:, 1]), "l1")
            chain(nc.sync.dma_start(xv3[:, 2], xv_r[:, 2]), "l2")

            wk3 = wkb[:, 0:NDT * FEAT].rearrange("p (t f) -> p t f", t=NDT)
            wq3 = wqb[:, 0:NDT * FEAT].rearrange("p (t f) -> p t f", t=NDT)
            bk3 = wkb[:, NDT * FEAT:NDT * FEAT + 4].bitcast(fp32)
            bq3 = wqb[:, NDT * FEAT:NDT * FEAT + 4].bitcast(fp32)
            wv3 = wvo[:, 0:NDT * FEAT].rearrange("p (t f) -> p t f", t=NDT)
            wo3 = wvo[:, NDT * FEAT:].rearrange("p (t j) -> p t j", t=2)

            # ---- persistent intermediates ----
            qh3 = proj.tile([128, 2, S], dt_qk, tag="qh")   # pair-packed
            kh3 = proj.tile([128, 2, S], dt_qk, tag="kh")
            vha = proj.tile([128, NKT, HL, DK + 1], dt_pv, tag="vha")
            ot3 = proj.tile([128, 2, S], dt_o, tag="outT")
            nc.gpsimd.memset(vha[:, :, :, DK], 1.0)  # ones col -> denominators

            # ---- PE p-state warmup while the first DMAs land: big matmuls
            # carry the clock ramp, then fine-grained N=128 fills keep the
            # PE hot (and the overshoot cheap) until xk_b0/wkb arrive ----
            wdum = wdum0
            wu = pp.tile([128, 512], fp32, tag="acc")
            for i in range(N_WARMUP):
                nc.tensor.matmul(wu[:], wdum[:, 0:128], wdum[:],
                                 start=(i == 0), stop=(i == N_WARMUP - 1))
            for i in range(N_WARMFILL):
                nc.tensor.matmul(wu[:, 0:128], wdum[:, 0:128],
                                 wdum[:, 0:128], start=True, stop=True)
            nc.vector.tensor_copy(junk[:], wu[:])

            # ---- projection quanta: one (pair m, seq-block blk) at a time,
            # kt-inner, chasing the per-block x DMAs ----
            def qk_blk(x3, w3, b3, dst, m, blk):
                acc = pp.tile([128, 512], fp32, tag="acc", name="acc")
                first = None
                for kt in range(NDT):
                    i = nc.tensor.matmul(
                        acc[:], w3[:, kt, m * 128:(m + 1) * 128],
                        x3[:, blk, kt, :],
                        start=(kt == 0), stop=(kt == NDT - 1))
                    first = first or i
                nc.vector.tensor_scalar_add(
                    dst[:, m, blk * 512:(blk + 1) * 512], acc[:],
                    b3[:, m:m + 1])
                return first

            def v_quantum(st):
                ps = pp.tile([128, 512], fp32, tag="acc", name="vacc")
                first = None
                for kt in range(NDT):
                    i = nc.tensor.matmul(
                        ps[:, 0:256],
                        xv3[:, st // 4, kt, (st % 4) * 128:(st % 4 + 1) * 128],
                        wv3[:, kt, :],
                        start=(kt == 0), stop=(kt == NDT - 1))
                    first = first or i
                nc.vector.tensor_copy(vha[:, st, :, 0:DK], ps[:, 0:256])
                return first

            def pv_quantum(state, qt, hp, e2u, kt):
                if "a" not in state:
                    state["a"] = pspv.tile([DK + 1, 512], fp32, tag="pv",
                                           name="pva")
                    state["b"] = pspv.tile([DK + 1, 512], fp32, tag="pv",
                                           name="pvb")
                i = nc.tensor.matmul(
                    state["a"][:], vha[:, kt, 2 * hp, :], e2u[:, kt, 0:512],
                    start=(kt == 0), stop=(kt == NKT - 1))
                nc.tensor.matmul(
                    state["b"][:], vha[:, kt, 2 * hp + 1, :],
                    e2u[:, kt, 512:1024],
                    start=(kt == 0), stop=(kt == NKT - 1))
                return i

            def norm(state, qt, hp, direct=False):
                # whole-accumulator copy frees the PSUM bank early; custom
                # DVE recip needs a base-partition-0 SBUF input (srow).
                # direct=True (final unit): skip the copy, read PSUM in
                # place — shorter critical chain, the bank isn't needed.
                for pv, half in ((state["a"], 0), (state["b"], 1)):
                    if direct:
                        pvs = pv
                    else:
                        pvs = pnrm.tile([DK + 1, 512], fp32, tag="pvs")
                        nc.vector.tensor_copy(pvs[:], pv[:])
                    srow = pnrm.tile([1, 512], fp32, tag="srow")
                    nc.vector.tensor_copy(srow[:], pvs[DK:DK + 1, :])
                    inv = pnrm.tile([1, 512], fp32, tag="inv")
                    nc.vector.reciprocal_approx_fast(inv[:], srow[:])
                    invb = pnrm.tile([64, 512], fp32, tag="invb")
                    nc.gpsimd.partition_broadcast(invb[:], inv[:])
                    nc.vector.tensor_tensor(
                        ot3[half * 64:(half + 1) * 64, hp,
                            qt * 512:(qt + 1) * 512],
                        pvs[0:DK, :], invb[:], mybir.AluOpType.mult)

            def oproj_quantum(pstate, qt, jt):
                if "po" not in pstate:
                    pstate["po"] = pout.tile([128, NDT, 512], dt_out,
                                             tag="po", bufs=1, name="po")
                ps = pp.tile([128, 512], fp32, tag="acc", name="oacc")
                first = None
                for m in range(2):
                    i = nc.tensor.matmul(
                        ps[:], wo3[:, m, jt * 128:(jt + 1) * 128],
                        ot3[:, m, qt * 512:(qt + 1) * 512],
                        start=(m == 0), stop=(m == 1))
                    first = first or i
                if qt == NQT - 1:
                    # ACT is idle once the exp stream ends; casting there
                    # overlaps the DVE normalize chain
                    nc.scalar.copy(pstate["po"][:, jt, :], ps[:])
                else:
                    nc.vector.tensor_copy(pstate["po"][:, jt, :], ps[:])
                if qt == NQT - 1:
                    # last tile: per-jt DMA starts the final drain earlier
                    nc.sync.dma_start(out_d[qt, :, jt:jt + 1, :],
                                      pstate["po"][:, jt:jt + 1, :])
                elif jt == NDT - 1:
                    nc.sync.dma_start(out_d[qt], pstate["po"][:])
                return first

            def e2tile(name):
                return big.tile([128, NKT, 1024], dt_pv, tag="big", name=name)

            # ---- filler queue: PE work that rides in the slack of the
            # exp-paced scores stream (ACT needs ~1088ns/kt, scores only
            # ~426ns of PE) so the exp stream never starves. Each released
            # filler's first matmul is dep-chained to the preceding scores
            # matmul: the Tile scheduler orders by its own cost model, and
            # without the explicit dep it hoists ready fillers wholesale
            # ahead of blocked scores (v3 lost 12us to exactly that). ----
            fillers = []      # list of (cost_ns, thunk)
            fq = {"i": 0, "budget": 0.0, "n": 0, "last": None}

            def pace(first_inst):
                # No explicit ordering edge: the Tile scheduler orders by
                # data readiness in its own cost-model sim, and measured
                # schedules show it interleaving fillers with the scores
                # stream near-optimally on its own. Explicit dep edges to
                # the last scores matmul were tried and cost ~40us: each
                # edge adds sequencer wait overhead and breaks the PE
                # queue's lookahead pipelining.
                pass

            def release_next():
                cost, fn = fillers[fq["i"]]
                fq["i"] += 1
                pace(fn())

            def drain_fillers():
                while fq["i"] < len(fillers):
                    release_next()

            SLACK_NS = 900    # filler budget added per exp-paced kt step

            def s_unit(qt, hp, e2u):
                for kt in range(NKT):
                    s2 = ps2.tile([128, 1024], fp32, tag="s2")
                    nc.tensor.matmul(
                        s2[:, 0:512],
                        kh3[0:64, hp, kt * 128:(kt + 1) * 128],
                        qh3[0:64, hp, qt * 512:(qt + 1) * 512],
                        start=True, stop=True)
                    fq["last"] = nc.tensor.matmul(
                        s2[:, 512:1024],
                        kh3[64:128, hp, kt * 128:(kt + 1) * 128],
                        qh3[64:128, hp, qt * 512:(qt + 1) * 512],
                        start=True, stop=True)
                    nc.scalar.activation(
                        e2u[:, kt, :], s2[:],
                        mybir.ActivationFunctionType.Exp, scale=0.125)
                    fq["budget"] += SLACK_NS
                    while (fq["i"] < len(fillers)
                           and fillers[fq["i"]][0] <= fq["budget"]):
                        fq["budget"] -= fillers[fq["i"]][0]
                        release_next()

            # ---- emission: k(0,b0)+q(0,b0) up front, then the exp-paced
            # score stream with everything else as fillers. The k fillers
            # alternate pair-1 (xk_b0-resident, ready immediately) with
            # pair-0 blocks (chasing the lane DMAs); each k(0,b) releases
            # BEFORE the scores step that reads it — both for correctness
            # of the pace() edge (no cycle) and so the stream only stalls
            # on the DMA itself. Queue order respects the e2-slot reuse
            # deps (all xk readers emitted before e01 takes xk3's slot,
            # xq readers before e10, xv readers before e11). ----
            qk_blk(xk3, wk3, bk3, kh3, 0, 0)
            qk_blk(xq3, wq3, bq3, qh3, 0, 0)

            for m, blk in ((1, 0), (0, 1), (1, 1), (0, 2), (1, 2), (0, 3),
                           (1, 3), (None, None)):
                if m is None:
                    fillers.append(
                        (1750, lambda: qk_blk(xq3, wq3, bq3, qh3, 1, 0)))
                else:
                    fillers.append(
                        (1750, lambda m=m, b=blk: qk_blk(
                            xk3, wk3, bk3, kh3, m, b)))

            e00 = e2tile("e00")
            s_unit(0, 0, e00)                        # ACT starts here
            drain_fillers()                          # xk readers, see above
            for m, blk in ((0, 1), (1, 1), (0, 2), (1, 2), (0, 3), (1, 3)):
                fillers.append(
                    (1750, lambda m=m, b=blk: qk_blk(
                        xq3, wq3, bq3, qh3, m, b)))
            e01 = e2tile("e01")
            s_unit(0, 1, e01)
            drain_fillers()                          # xq readers
            for st in range(NKT):                    # v-proj -> fillers
                fillers.append((900, lambda st=st: v_quantum(st)))
            e10 = e2tile("e10")
            s_unit(1, 0, e10)
            drain_fillers()                          # xv readers
            pv_states = {}
            for u, (uq, uh, eu) in enumerate(((0, 0, e00), (0, 1, e01))):
                st_ = pv_states[(uq, uh)] = {}
                for kt in range(NKT):
                    fillers.append(
                        (440, lambda s=st_, q=uq, h=uh, e=eu, k=kt:
                         pv_quantum(s, q, h, e, k)))
                fillers.append(
                    (0, lambda s=st_, q=uq, h=uh: norm(s, q, h)))
            e11 = e2tile("e11")
            s_unit(1, 1, e11)

            prev = {(1, 0): e10, (1, 1): e11}
            def queue_pv(qt, hp):
                st_ = pv_states[(qt, hp)] = {}
                eu = prev[(qt, hp)]
                for kt in range(NKT):
                    fillers.append(
                        (440, lambda s=st_, q=qt, h=hp, e=eu, k=kt:
                         pv_quantum(s, q, h, e, k)))
                fillers.append(
                    (0, lambda s=st_, q=qt, h=hp: norm(s, q, h)))

            def queue_oproj(qt):
                pstate = {}
                for jt in range(NDT):
                    fillers.append(
                        (480, lambda p=pstate, q=qt, j=jt:
                         oproj_quantum(p, q, j)))

            # one PV unit (+ one oproj set) per remaining window, so the
            # filler queue inflow matches the ~14us/window release budget
            queue_pv(1, 0)
            queue_pv(1, 1)
            e20 = e2tile("e20")
            prev[(2, 0)] = e20
            s_unit(2, 0, e20)
            queue_pv(2, 0)
            queue_oproj(0)
            e21 = e2tile("e21")
            prev[(2, 1)] = e21
            s_unit(2, 1, e21)
            queue_pv(2, 1)
            queue_oproj(1)
            e30 = e2tile("e30")
            prev[(3, 0)] = e30
            s_unit(3, 0, e30)
            queue_pv(3, 0)
            queue_oproj(2)
            e31 = e2tile("e31")
            prev[(3, 1)] = e31
            s_unit(3, 1, e31)

            # tail: leftover fillers, last PV tracking the last exps, then
            # o-proj(3) pair-0 matmuls pre-issued (plus zero-accumulate
            # keepers) bridge the final normalize so the PE stays hot and
            # only the pair-1 matmuls trail the last ot3 write
            drain_fillers()
            st_ = pv_states[(3, 1)] = {}
            for kt in range(NKT):
                pv_quantum(st_, 3, 1, prev[(3, 1)], kt)
            q3s = slice((NQT - 1) * 512, NQT * 512)
            accs = []
            for jt in range(6):
                if jt % 2 == 0 and jt < 4:
                    ot_ps = ps2.tile([128, 1024], fp32, tag="s2",
                                     name=f"otps{jt // 2}")
                if jt < 4:
                    acc = ot_ps[:, (jt % 2) * 512:(jt % 2 + 1) * 512]
                else:
                    acc = pp.tile([128, 512], fp32, tag="acc",
                                  name=f"oacc{jt}")[:]
                nc.tensor.matmul(acc, wo3[:, 0, jt * 128:(jt + 1) * 128],
                                 ot3[:, 0, q3s], start=True, stop=False)
                accs.append(acc)
            norm(st_, 3, 1)
            for acc in accs:      # zero-accumulate keepers during the norm
                nc.tensor.matmul(acc, wdum[:, 0:128], wdum[:],
                                 start=False, stop=False)
            po = pout.tile([128, NDT, 512], dt_out, tag="po", bufs=1,
                           name="po")
            for jt in range(6):
                nc.tensor.matmul(accs[jt],
                                 wo3[:, 1, jt * 128:(jt + 1) * 128],
                                 ot3[:, 1, q3s], start=False, stop=True)
                nc.scalar.copy(po[:, jt, :], accs[jt])
                nc.sync.dma_start(out_d[NQT - 1, :, jt:jt + 1, :],
                                  po[:, jt:jt + 1, :])
            pstate = {"po": po}
            for jt in (6, 7):
                oproj_quantum(pstate, 3, jt)

    nc.compile()
    return nc


def kernel(q, k, v, Wq, bq, Wk, bk, Wv, bv, Wo, bo, _trace=False):
    from concourse import bass_utils

    if "nc" not in _cache:
        _cache["nc"] = _build()
    nc = _cache["nc"]

    q = np.asarray(q, np.float32)
    k = np.asarray(k, np.float32)
    v = np.asarray(v, np.float32)
    Wq = np.asarray(Wq, np.float32)
    Wk = np.asarray(Wk, np.float32)
    Wv = np.asarray(Wv, np.float32)
    Wo = np.asarray(Wo, np.float32)
    bq = np.asarray(bq, np.float32)
    bk = np.asarray(bk, np.float32)
    bv = np.asarray(bv, np.float32)
    bo = np.asarray(bo, np.float32)

    d_qk, d_v, d_o = _np_dt(DT_QK), _np_dt(DT_V), _np_dt(DT_O)

    def swz(a, t):     # [t*128, f] -> [128, t*f], rows contiguous in DRAM
        f = a.shape[1]
        return np.ascontiguousarray(
            a.reshape(t, 128, f).transpose(1, 0, 2).reshape(128, t * f))

    def swz_seq(a):    # [D, S] -> [128, (blk, kt, 512)] seq-major blocks
        # a[t*128+p, blk*512+s] -> out[p, ((blk*NDT)+t)*512+s]
        return np.ascontiguousarray(
            a.reshape(NDT, 128, NQT, 512).transpose(1, 2, 0, 3)
            .reshape(128, NQT * NDT * 512))

    xT = {}
    for b in range(B):
        xT[("q", b)] = swz_seq(np.ascontiguousarray(q[b].T)).astype(d_qk)
        xT[("k", b)] = swz_seq(np.ascontiguousarray(k[b].T)).astype(d_qk)
        xT[("v", b)] = swz_seq(np.ascontiguousarray(v[b].T)).astype(d_v)
    wT = {}
    for g in range(HG):
        sl = slice(g * FEAT, (g + 1) * FEAT)
        wk_s = swz(np.ascontiguousarray(Wk[sl, :].T), NDT).astype(d_qk)
        wq_s = swz(np.ascontiguousarray(Wq[sl, :].T), NDT).astype(d_qk)
        bk_s = np.ascontiguousarray(
            bk[sl].astype(np.float32).reshape(2, 128).T).view(np.uint16)
        bq_s = np.ascontiguousarray(
            bq[sl].astype(np.float32).reshape(2, 128).T).view(np.uint16)
        wv_s = swz(np.ascontiguousarray(Wv[sl, :].T), NDT).astype(d_v)
        wo_s = swz(np.ascontiguousarray(Wo[:, sl].T), 2).astype(d_o)
        wT[("kb", g)] = np.ascontiguousarray(np.concatenate(
            [wk_s.view(np.uint16), bk_s], axis=1).view(d_qk))
        wT[("qb", g)] = np.ascontiguousarray(np.concatenate(
            [wq_s.view(np.uint16), bq_s], axis=1).view(d_qk))
        wT[("vo", g)] = np.ascontiguousarray(
            np.concatenate([wv_s, wo_s], axis=1))

    in_maps = []
    for c in range(N_CORES):
        b, g = divmod(c, HG)
        in_maps.append({
            "xqT": xT[("q", b)], "xkT": xT[("k", b)], "xvT": xT[("v", b)],
            "wkbT": wT[("kb", g)], "wqbT": wT[("qb", g)],
            "wvoT": wT[("vo", g)],
        })

    kwargs = {}
    if _trace:
        _install_profile_shim()
        kwargs = dict(trace=True, trace_cores=list(range(N_CORES)))
    res = bass_utils.run_bass_kernel_spmd(
        nc, in_maps, core_ids=list(range(N_CORES)), **kwargs)
    _cache["last_results"] = res

    final_bias = (Wo @ bv + bo).astype(np.float32)  # attn rows sum to 1
    out = np.empty((B, S, D), np.float32)
    for b in range(B):
        acc = res.results[b * HG]["partialT"].astype(np.float32)
        for g in range(1, HG):
            acc += res.results[b * HG + g]["partialT"].astype(np.float32)
        # [qt, p, jt, s] -> [S, D]:  d = jt*128+p, q = qt*512+s
        out[b] = acc.transpose(0, 3, 2, 1).reshape(S, D) + final_bias
    return out


def _install_profile_shim():
    """Provide antenv.axon_hooks so trace=True works under axon."""
    import sys
    import types

    import antenv

    if "antenv.axon_hooks" in sys.modules:
        return
    mod = types.ModuleType("antenv.axon_hooks")
    mod._hook = None
    mod.set_axon_ntff_profile_hook = lambda h: setattr(mod, "_hook", h)
    mod.get_axon_ntff_profile_hook = lambda: mod._hook
    sys.modules["antenv.axon_hooks"] = mod
    antenv.axon_hooks = mod
    try:
        from trn_agent_boot.trn_boot import _ntff_profile_via_ctypes
        mod.set_axon_ntff_profile_hook(
            _ntff_profile_via_ctypes("/opt/axon/libaxon_pjrt.so"))
    except Exception:
        pass
